# revision 13
# baseline (speedup 1.0000x reference)
"""MANN (phase-blended mixture-of-experts) forward pass on 8 Trainium2 cores.

Strategy (data-parallel, per sharding hint):
  - Shard batch B=512 across 8 cores (64 samples each); replicate all weights.
  - Host-side prep: transpose expert weights to [K, IN, OUT] (so the device
    streams them in natural layout with the contraction dim on partitions),
    pad layer-1 input dim 480 -> 512, pre-gather the gating columns.
  - Device: activations kept transposed [feat, B].  Key algebraic trick:
        y = sum_k g[:,k] * (x @ Wk[k].T)  ==  sum_k ((g[:,k]*x) @ Wk[k].T)
    so scaling the stationary activations by g[:,k] lets all 8 experts x 4
    K-subtiles accumulate into a single PSUM tile per layer.  The blended
    bias g @ bk is one extra small matmul into the same PSUM group.
  - ELU built from primitives: elu(x) = max(x, exp(min(x,0)) - 1).

Modes (MANN_MM_MODE env var, default bf16):
  bf16  - weights cast to bf16 on host (rel err ~3.5e-3 vs the 2e-2 gate):
          halves DMA traffic vs fp32 and runs matmuls at 1 cycle/row.
  fp32  - exact (rel err ~6e-7) but ~3.5x slower (4 cycles/row + 2x DMA).

Weight slabs are stored host-side partition-major ([K, P, KSUB*OUT], 4KB
contiguous per partition) -- this DMA-descriptor layout change took the
8-core steady-state body from ~106us to ~49us (the [ko p] strided layout
was descriptor-/row-thrash-limited, not HBM-bandwidth-limited).

All 8 cores share one trn2 chip (topology trn2.8x1, LNC=1), so replicated
weights stream 8x through one HBM: ~93MB/body at bf16.  An expert-sharded
variant (MANN_IMPL=pair: 4 experts/core, batch across 4 SEngine pairs,
SBUF->SBUF remote_dma partial exchange) would halve that and was fully
built + validated in the multi-core simulator, but still hits an opaque
device-side failure under the axon PJRT path -- kept env-gated off.
(float32r was investigated and rejected: walrus requires an fp32->fp32r
data conversion that the host API does not expose.)
"""

import json
import os

import numpy as np
import ml_dtypes

import concourse.bass as bass
import concourse.bass2jax as bass2jax
import concourse.mybir as mybir
import concourse.tile as tile
from concourse import bass_utils as _bass_utils
from concourse.bass_utils import run_bass_kernel_spmd
from concourse.masks import make_identity


# ---------------------------------------------------------------------------
# Post-scheduling BIR wait injection.
#
# Cross-core remote_dma semaphore increments are invisible to the Tile
# scheduling simulator (single-core, no-exec), so an in-trace wait_ge on a
# remotely-incremented semaphore deadlocks the scheduler.  Instead the trace
# carries no such waits; we record (instruction name -> waits) in a global
# table keyed by a sentinel semaphore name placed in the module, and inject
# the sync_info entries into the serialized BIR right before compilation.
# Tile's own RAW/WAR edges on the remote-DMA descgen instructions (whose
# outs alias the local recv buffers) keep program order correct; the
# injected waits provide the actual data-arrival gate at runtime.
_WJ_TABLES = {}


class WaitInjector:
    def __init__(self, nc):
        import uuid
        self.key = f"wjtag_{uuid.uuid4().hex[:12]}"
        nc.alloc_semaphore(name=self.key)  # sentinel: marks the module
        self.table = _WJ_TABLES.setdefault(self.key, {})

    def add(self, inst, sem, value):
        self.table.setdefault(inst.ins.name, []).append(
            (sem.name, sem.num, int(value)))


def _inject_waits(data):
    names = {n for ns in data.get("ant_sem_names", {}).values() for n in ns}
    key = next((n for n in names if n.startswith("wjtag_")), None)
    table = _WJ_TABLES.get(key)
    if not table:
        return data
    hit = 0
    for fn in data.get("functions", []):
        for bb in fn.get("blocks", []):
            for inst in bb.get("instructions", []):
                ws = table.get(inst.get("name"))
                if not ws:
                    continue
                hit += 1
                si = inst.get("sync_info")
                if si is None:
                    si = {"on_update": [], "on_wait": []}
                    inst["sync_info"] = si
                for sname, sid, val in ws:
                    si.setdefault("on_wait", []).append({
                        "ant_name": sname, "id": sid,
                        "sync_type": "semaphore",
                        "wait_mode": "sem-ge-imm", "wait_value": val,
                    })
    assert hit == len(table), (
        f"wait injection: only {hit}/{len(table)} tagged instructions "
        f"found in BIR"
    )
    return data


def _legalize_bir(bir_bytes):
    """This container's walrus build rejects instructions carrying more than
    one semaphore wait (setupSyncWait: "Too many sync wait commands" -- hit by
    the Tile kernel-tail Drain).  Equivalent legal form: hoist all but one
    wait onto single-wait NoOps immediately preceding the instruction on the
    same engine (sequencers process waits in program order)."""
    data = _inject_waits(json.loads(bir_bytes))
    n = 0
    for fn in data.get("functions", []):
        for bb in fn.get("blocks", []):
            out = []
            for inst in bb.get("instructions", []):
                si = inst.get("sync_info")
                waits = si.get("on_wait", []) if si else []
                if len(waits) > 1:
                    for w in waits[:-1]:
                        n += 1
                        out.append({
                            "debug": inst.get("debug", 0),
                            "engine": inst["engine"],
                            "ins": [], "outs": [],
                            "name": f"I-mwfix-{n}",
                            "opcode": "NoOp",
                            "sync_info": {"on_update": [], "on_wait": [w]},
                        })
                    si["on_wait"] = [waits[-1]]
                out.append(inst)
            bb["instructions"] = out
    return json.dumps(data).encode()


_orig_compile_bir_kernel = _bass_utils.compile_bir_kernel


def _patched_compile_bir_kernel(bir_json, tmpdir, neff_name="file.neff"):
    return _orig_compile_bir_kernel(_legalize_bir(bir_json), tmpdir,
                                    neff_name=neff_name)


bass2jax.compile_bir_kernel = _patched_compile_bir_kernel
_bass_utils.compile_bir_kernel = _patched_compile_bir_kernel

B, IN_DIM, OUT_DIM, HID, K, GH, NG = 512, 480, 400, 512, 8, 128, 32
N_CORES = 8
BS = B // N_CORES  # 64 samples per core
IN_PAD = 512       # layer-1 contraction dim padded to 4x128
KSUB = 4           # 512 / 128 contraction subtiles (all layers, post-pad)
OUTS = (HID, HID, OUT_DIM)
P = 128

MM_MODE = os.environ.get("MANN_MM_MODE", "bf16")

# Set to the BassKernelResults of the last run (for test harnesses).
LAST_RESULTS = None

_NC_CACHE = {}


def _elu_from(nc, pool, src_ap, out_shape, tag):
    """elu(src) = max(src, min(exp(src), 1) - 1); src may be PSUM or SBUF.
    3 ops, exp directly from src (activations here are small enough that
    exp cannot overflow fp32).  Returns a new SBUF fp32 tile."""
    f32 = mybir.dt.float32
    texp = pool.tile(out_shape, f32, tag=f"{tag}_exp")
    nc.scalar.activation(texp, src_ap, mybir.ActivationFunctionType.Exp)
    nc.vector.tensor_scalar(texp, texp, 1.0, -1.0, mybir.AluOpType.min,
                            mybir.AluOpType.add)
    y = pool.tile(out_shape, f32, tag=f"{tag}_y")
    nc.vector.tensor_tensor(y, src_ap, texp, mybir.AluOpType.max)
    return y


def _build(mode, repeat=1):
    f32 = mybir.dt.float32
    if mode == "bf16":
        wdt = mybir.dt.bfloat16
        mmdt = mybir.dt.bfloat16
    else:
        wdt = f32
        mmdt = f32

    def mm_ap(ap):
        return ap

    nc = bass.Bass()

    xT_d = nc.dram_tensor("xT", [IN_PAD, BS], f32, kind="ExternalInput")
    ginT_d = nc.dram_tensor("ginT", [NG, BS], f32, kind="ExternalInput")
    # Weight slabs stored host-side as [K, P, KSUB*OUT] (partition-major,
    # 4KB bf16 contiguous per partition) so each expert slab is one DMA of
    # 128 large contiguous descriptors instead of 512 strided 1KB ones.
    w_d = [
        nc.dram_tensor(f"w{l}", [K, P, KSUB * OUTS[l]], wdt,
                       kind="ExternalInput")
        for l in range(3)
    ]
    b_d = [
        nc.dram_tensor(f"b{l}", [K, OUTS[l]], wdt, kind="ExternalInput")
        for l in range(3)
    ]
    gw1_d = nc.dram_tensor("gw1", [NG, GH], f32, kind="ExternalInput")
    gw2_d = nc.dram_tensor("gw2", [GH, GH], f32, kind="ExternalInput")
    gw3_d = nc.dram_tensor("gw3", [GH, K], f32, kind="ExternalInput")
    gb1_d = nc.dram_tensor("gb1", [GH, 1], f32, kind="ExternalInput")
    gb2_d = nc.dram_tensor("gb2", [GH, 1], f32, kind="ExternalInput")
    gb3_d = nc.dram_tensor("gb3", [K, 1], f32, kind="ExternalInput")
    # E[j, e*128 + p] = (j == e): replicates g row e across 128 partitions
    # via matmul E_slice.T @ gT.
    emat_d = nc.dram_tensor("emat", [K, K * P], f32, kind="ExternalInput")
    out_d = nc.dram_tensor("out", [BS, OUT_DIM], f32, kind="ExternalOutput")

    w_bufs = int(os.environ.get("MANN_W_BUFS", "24" if mode == "bf16" else "12"))
    with tile.TileContext(nc) as tc:
        with (
            tc.tile_pool(name="consts", bufs=1) as cpool,
            tc.tile_pool(name="w", bufs=w_bufs) as wpool,
            tc.tile_pool(name="stat", bufs=2) as spool,
            tc.tile_pool(name="xt", bufs=2) as xpool,
            tc.tile_pool(name="y", bufs=2) as ypool,
            tc.tile_pool(name="psy", bufs=2, space="PSUM") as pspool,
            tc.tile_pool(name="pstr", bufs=2, space="PSUM") as ptpool,
            tc.tile_pool(name="psg", bufs=1, space="PSUM") as pgpool,
        ):
            pools = (cpool, wpool, spool, xpool, ypool, pspool, ptpool, pgpool)

            # ---- constants ----
            xt0 = cpool.tile([P, KSUB, BS], f32)
            nc.sync.dma_start(xt0, xT_d.rearrange("(ko p) b -> p ko b", p=P))
            gin = cpool.tile([NG, BS], f32)
            nc.sync.dma_start(gin, ginT_d[:])
            gw1 = cpool.tile([NG, GH], f32)
            nc.sync.dma_start(gw1, gw1_d[:])
            gw2 = cpool.tile([GH, GH], f32)
            nc.sync.dma_start(gw2, gw2_d[:])
            gw3 = cpool.tile([GH, K], f32)
            nc.sync.dma_start(gw3, gw3_d[:])
            gb1 = cpool.tile([GH, 1], f32)
            nc.sync.dma_start(gb1, gb1_d[:])
            gb2 = cpool.tile([GH, 1], f32)
            nc.sync.dma_start(gb2, gb2_d[:])
            gb3 = cpool.tile([K, 1], f32)
            nc.sync.dma_start(gb3, gb3_d[:])
            emat = cpool.tile([K, K * P], f32)
            nc.sync.dma_start(emat, emat_d[:])
            bts = []
            for l in range(3):
                bt = cpool.tile([K, OUTS[l]], wdt, tag=f"b{l}")
                nc.sync.dma_start(bt, b_d[l][:])
                bts.append(bt)
            ident = cpool.tile([BS, BS], f32)
            make_identity(nc, ident)
            consts = (xt0, gin, gw1, gw2, gw3, gb1, gb2, gb3, emat, bts, ident)

            if repeat == 0:
                # no-op baseline for dispatch-overhead measurement
                yo = ypool.tile([BS, OUT_DIM], f32, tag="yo")
                nc.vector.memset(yo, 0.0)
                nc.sync.dma_start(out_d[:], yo)
            for _rep in range(repeat):
                _emit_body(nc, mode, mmdt, mm_ap, wdt, pools, w_d, b_d, out_d,
                           consts, accum=(_rep > 0))

    return nc


def _emit_body(nc, mode, mmdt, mm_ap, wdt, pools, w_d, b_d, out_d, consts,
               accum=False):
    f32 = mybir.dt.float32
    cpool, wpool, spool, xpool, ypool, pspool, ptpool, pgpool = pools
    xt0, gin, gw1, gw2, gw3, gb1, gb2, gb3, emat, bts, ident = consts

    # ---- weight slab DMAs, issued first (DMA is the bottleneck) ----
    wsl = []
    for l in range(3):
        row = []
        for e in range(K):
            t = wpool.tile([P, KSUB, OUTS[l]], wdt, tag="w")
            nc.sync.dma_start(
                t[:, :, : OUTS[l]],
                w_d[l][e].rearrange("p (a b) -> p a b", a=KSUB),
            )
            row.append(t)
        wsl.append(row)

    # ---- gating MLP (fp32, exact) ----
    pg1 = pgpool.tile([GH, BS], f32, tag="psg")
    nc.tensor.matmul(pg1, lhsT=gw1, rhs=gin, start=True, stop=True)
    zg1 = ypool.tile([GH, BS], f32, tag="zg1")
    nc.scalar.activation(zg1, pg1, mybir.ActivationFunctionType.Identity,
                         bias=gb1)
    h1 = _elu_from(nc, ypool, zg1, [GH, BS], "g1")

    pg2 = pgpool.tile([GH, BS], f32, tag="psg")
    nc.tensor.matmul(pg2, lhsT=gw2, rhs=h1, start=True, stop=True)
    zg2 = ypool.tile([GH, BS], f32, tag="zg2")
    nc.scalar.activation(zg2, pg2, mybir.ActivationFunctionType.Identity,
                         bias=gb2)
    h2 = _elu_from(nc, ypool, zg2, [GH, BS], "g2")

    pg3 = pgpool.tile([K, BS], f32, tag="psg")
    nc.tensor.matmul(pg3, lhsT=gw3, rhs=h2, start=True, stop=True)
    gT = ypool.tile([K, BS], f32, tag="gT")
    nc.scalar.activation(gT, pg3, mybir.ActivationFunctionType.Identity,
                         bias=gb3)
    if mode == "bf16":
        gT_mm = ypool.tile([K, BS], mmdt, tag="gTmm")
        nc.vector.tensor_copy(gT_mm, gT)
    else:
        gT_mm = gT

    # replicate g across partitions: gTb[p, e, b] = g[b, e]
    pgt = pgpool.tile([P, K, BS], f32, tag="psgtb")
    for e in range(K):
        nc.tensor.matmul(pgt[:, e, :], lhsT=emat[:, e * P:(e + 1) * P],
                         rhs=gT, start=True, stop=True)
    gTb = ypool.tile([P, K, BS], f32, tag="gTb")
    nc.vector.tensor_copy(gTb, pgt)

    # ---- motion layers ----
    # Each layer's output columns are split into two halves so the DVE/ACT
    # post-processing (ELU) and PE transposes of half 0 overlap the PE
    # matmuls of half 1.
    xt = xt0
    sdt = mmdt if mode == "bf16" else f32
    for l in range(3):
        outl = OUTS[l]
        halves = [(0, 256), (256, outl)]

        # per-expert scaled stationaries: one broadcast mult per (expert,
        # k-half) instead of 32 tiny mults
        xk = spool.tile([P, K, KSUB, BS], sdt, tag="xk")
        for e in range(K):
            gslab = gTb[:, e:e + 1, :].to_broadcast((P, 2, BS))
            nc.vector.tensor_tensor(xk[:, e, 0:2, :], xt[:, 0:2, :], gslab,
                                    mybir.AluOpType.mult)
            nc.vector.tensor_tensor(xk[:, e, 2:4, :], xt[:, 2:4, :], gslab,
                                    mybir.AluOpType.mult)

        use_pair = os.environ.get("MANN_PAIR", "1") == "1"
        pss = []
        for h, (lo, hi) in enumerate(halves):
            if use_pair:
                # Two experts run concurrently in disjoint 64-col groups of
                # the PE array (even experts -> psum rows 0:64, odd ->
                # 64:128 via tile_position=(0,64)); summed on DVE after.
                ps_full = pspool.tile([2 * BS, 256], f32, tag=f"psy{h}",
                                      name=f"psy{l}_{h}")
                psA = ps_full[0:BS, : hi - lo]
                psB = ps_full[BS:2 * BS, : hi - lo]
                nc.tensor.matmul(psA, lhsT=mm_ap(gT_mm),
                                 rhs=mm_ap(bts[l][:, lo:hi]),
                                 start=True, stop=False,
                                 skip_group_check=True)
                for e0 in range(0, K, 2):
                    for ks in range(KSUB):
                        last = (e0 == K - 2 and ks == KSUB - 1)
                        nc.tensor.matmul(
                            psA,
                            lhsT=mm_ap(xk[:, e0, ks, :]),
                            rhs=mm_ap(wsl[l][e0][:, ks, lo:hi]),
                            start=False, stop=last,
                            skip_group_check=True,
                        )
                        nc.tensor.matmul(
                            psB,
                            lhsT=mm_ap(xk[:, e0 + 1, ks, :]),
                            rhs=mm_ap(wsl[l][e0 + 1][:, ks, lo:hi]),
                            start=(e0 == 0 and ks == 0), stop=last,
                            tile_position=(0, BS),
                            skip_group_check=True,
                        )
                pss.append((psA, psB))
            else:
                ps_full = pspool.tile([BS, 256], f32, tag=f"psy{h}",
                                      name=f"psy{l}_{h}")
                ps = ps_full[:, : hi - lo]
                nc.tensor.matmul(ps, lhsT=mm_ap(gT_mm),
                                 rhs=mm_ap(bts[l][:, lo:hi]),
                                 start=True, stop=False)
                for e in range(K):
                    for ks in range(KSUB):
                        nc.tensor.matmul(
                            ps,
                            lhsT=mm_ap(xk[:, e, ks, :]),
                            rhs=mm_ap(wsl[l][e][:, ks, lo:hi]),
                            start=False,
                            stop=(e == K - 1 and ks == KSUB - 1),
                        )
                pss.append((ps, None))

        if l < 2:
            ptr = ptpool.tile([P, KSUB, BS], f32, tag="ptr")
            xt_next = xpool.tile([P, KSUB, BS], f32, tag="xtn")
            for h, (lo, hi) in enumerate(halves):
                psA, psB = pss[h]
                if psB is not None:
                    # DVE may read only one PSUM operand per instruction:
                    # copy psB to SBUF first, then add.
                    zb = ypool.tile([BS, hi - lo], f32, tag=f"zb{h}")
                    nc.vector.tensor_copy(zb, psB)
                    z = ypool.tile([BS, hi - lo], f32, tag=f"z{h}")
                    nc.vector.tensor_tensor(z, psA, zb,
                                            mybir.AluOpType.add)
                    src = z
                else:
                    src = psA
                y = _elu_from(nc, ypool, src, [BS, hi - lo], f"ml{h}")
                for c in range(2):
                    nc.tensor.transpose(ptr[:, 2 * h + c, :],
                                        y[:, c * P:(c + 1) * P], ident)
                nc.vector.tensor_copy(xt_next[:, 2 * h:2 * h + 2, :],
                                      ptr[:, 2 * h:2 * h + 2, :])
            xt = xt_next
        else:
            yo = ypool.tile([BS, OUT_DIM], f32, tag="yo")
            for h, (lo, hi) in enumerate(halves):
                psA, psB = pss[h]
                if psB is not None:
                    zb = ypool.tile([BS, hi - lo], f32, tag=f"zb{h}")
                    nc.vector.tensor_copy(zb, psB)
                    nc.vector.tensor_tensor(yo[:, lo:hi], psA, zb,
                                            mybir.AluOpType.add)
                else:
                    nc.vector.tensor_copy(yo[:, lo:hi], psA)
            if accum:
                # benchmark-repeat builds accumulate so no body is dead code
                nc.gpsimd.dma_start(out_d[:], yo,
                                    accum_op=mybir.AluOpType.add)
            else:
                nc.sync.dma_start(out_d[:], yo)


# ---------------------------------------------------------------------------
# Pair-sharded implementation (MANN_IMPL=pair).
#
# 8 cores = 4 SEngine pairs.  Within a pair, the 8 experts are split 4/4;
# the batch is sharded across pairs (128 samples each).  Each core computes
# the partial blend over its 4 experts for the pair's 128 samples, the two
# partials are exchanged SBUF->SBUF over the intra-SEngine link with one
# remote_dma_broadcast per layer, and both cores reduce + elu.  Activations
# stay in [feature, batch] orientation the whole way (weights are the
# stationary operand) so no transposes are needed.  Total HBM weight
# traffic halves vs full replication (each expert is read by 4 cores, not
# 8).  Cross-core waits are injected post-scheduling (see WaitInjector).
E_PAIR = 4          # experts per core
BSP = 128           # samples per pair
OUT_PAD = 512       # L3 out dim padded 400 -> 512


def _elu_bf16(nc, pool, src_ap, shape, tag):
    """elu with bf16 output tile."""
    f32 = mybir.dt.float32
    texp = pool.tile(shape, f32, tag=f"{tag}_exp")
    nc.scalar.activation(texp, src_ap, mybir.ActivationFunctionType.Exp)
    nc.vector.tensor_scalar(texp, texp, 1.0, -1.0, mybir.AluOpType.min,
                            mybir.AluOpType.add)
    y = pool.tile(shape, mybir.dt.bfloat16, tag=f"{tag}_y")
    nc.vector.tensor_tensor(y, src_ap, texp, mybir.AluOpType.max)
    return y


def _build_pair(repeat=1):
    import concourse.tile as tile_mod
    from concourse import library_config
    from concourse.library_overlay import lower_extended_insts

    f32 = mybir.dt.float32
    bf16 = mybir.dt.bfloat16
    nc = bass.Bass()

    xTb_d = nc.dram_tensor("xTb", [P, KSUB, BSP], bf16, kind="ExternalInput")
    gin_d = nc.dram_tensor("ginT", [NG, BSP], bf16, kind="ExternalInput")
    gw1_d = nc.dram_tensor("gw1", [NG, GH], bf16, kind="ExternalInput")
    gw2_d = nc.dram_tensor("gw2", [GH, GH], bf16, kind="ExternalInput")
    gw3_d = nc.dram_tensor("gw3", [GH, K], bf16, kind="ExternalInput")
    gb1_d = nc.dram_tensor("gb1", [GH, 1], f32, kind="ExternalInput")
    gb2_d = nc.dram_tensor("gb2", [GH, 1], f32, kind="ExternalInput")
    gb3_d = nc.dram_tensor("gb3", [K, 1], f32, kind="ExternalInput")
    selr_d = nc.dram_tensor("selr", [K, E_PAIR * P], bf16,
                            kind="ExternalInput")
    w_d = [nc.dram_tensor(f"w{l}", [E_PAIR, P, KSUB * OUT_PAD], bf16,
                          kind="ExternalInput") for l in range(3)]
    bk_d = [nc.dram_tensor(f"bk{l}", [K, KSUB, P], bf16,
                           kind="ExternalInput") for l in range(3)]
    out_d = nc.dram_tensor("out", [P, KSUB * BSP], f32, kind="ExternalOutput")

    wj = WaitInjector(nc)
    NBUF = 3  # exchange-buffer rotation depth
    rsem = [nc.alloc_semaphore(name=f"xsemr{i}") for i in range(NBUF)]
    lsem = nc.alloc_semaphore(name="xseml")

    w_bufs = int(os.environ.get("MANN_PAIR_WBUFS", "24"))
    with tile_mod.TileContext(nc) as tc:
        with (
            tc.tile_pool(name="consts", bufs=1) as cpool,
            tc.tile_pool(name="w", bufs=w_bufs) as wpool,
            tc.tile_pool(name="xk", bufs=2) as xkpool,
            tc.tile_pool(name="y", bufs=3) as ypool,
            tc.tile_pool(name="ps", bufs=2, space="PSUM") as pspool,
            tc.tile_pool(name="psg", bufs=2, space="PSUM") as pgpool,
        ):
            nc.gpsimd.load_library(library_config.proxy)
            xTb = cpool.tile([P, KSUB, BSP], bf16)
            nc.sync.dma_start(xTb, xTb_d[:])
            gin = cpool.tile([NG, BSP], bf16)
            nc.sync.dma_start(gin, gin_d[:])
            gw1 = cpool.tile([NG, GH], bf16)
            nc.sync.dma_start(gw1, gw1_d[:])
            gw2 = cpool.tile([GH, GH], bf16)
            nc.sync.dma_start(gw2, gw2_d[:])
            gw3 = cpool.tile([GH, K], bf16)
            nc.sync.dma_start(gw3, gw3_d[:])
            gb1 = cpool.tile([GH, 1], f32)
            nc.sync.dma_start(gb1, gb1_d[:])
            gb2 = cpool.tile([GH, 1], f32)
            nc.sync.dma_start(gb2, gb2_d[:])
            gb3 = cpool.tile([K, 1], f32)
            nc.sync.dma_start(gb3, gb3_d[:])
            selr = cpool.tile([K, E_PAIR * P], bf16)
            nc.sync.dma_start(selr, selr_d[:])
            bks = []
            for l in range(3):
                bk = cpool.tile([K, KSUB, P], bf16, tag=f"bk{l}")
                nc.sync.dma_start(bk, bk_d[l][:])
                bks.append(bk)
            sendb = cpool.tile([P, NBUF, KSUB, BSP], bf16, name="sendb")
            recvb = cpool.tile([P, NBUF, KSUB, BSP], bf16, name="recvb")

            occ = 0
            for _rep in range(repeat):
                # --- weight DMAs first (the dominant HBM stream) ---
                wsl = []
                for l in range(3):
                    row = []
                    for e in range(E_PAIR):
                        t = wpool.tile([P, KSUB, OUT_PAD], bf16, tag="w")
                        nc.sync.dma_start(
                            t, w_d[l][e].rearrange("p (a b) -> p a b",
                                                   a=KSUB))
                        row.append(t)
                    wsl.append(row)

                # --- gating MLP on the pair's 128 samples (bf16) ---
                pg1 = pgpool.tile([GH, BSP], f32, tag="pg")
                nc.tensor.matmul(pg1, lhsT=gw1, rhs=gin, start=True,
                                 stop=True)
                zg1 = ypool.tile([GH, BSP], f32, tag="zg1")
                nc.scalar.activation(zg1, pg1,
                                     mybir.ActivationFunctionType.Identity,
                                     bias=gb1)
                h1 = _elu_bf16(nc, ypool, zg1, [GH, BSP], "g1")
                pg2 = pgpool.tile([GH, BSP], f32, tag="pg")
                nc.tensor.matmul(pg2, lhsT=gw2, rhs=h1, start=True,
                                 stop=True)
                zg2 = ypool.tile([GH, BSP], f32, tag="zg2")
                nc.scalar.activation(zg2, pg2,
                                     mybir.ActivationFunctionType.Identity,
                                     bias=gb2)
                h2 = _elu_bf16(nc, ypool, zg2, [GH, BSP], "g2")
                pg3 = pgpool.tile([K, BSP], f32, tag="pg")
                nc.tensor.matmul(pg3, lhsT=gw3, rhs=h2, start=True,
                                 stop=True)
                gT16 = ypool.tile([K, BSP], bf16, tag="gT16")
                nc.scalar.activation(gT16, pg3,
                                     mybir.ActivationFunctionType.Identity,
                                     bias=gb3)
                # replicate my 4 experts' g rows across all 128 partitions
                psel = pgpool.tile([P, E_PAIR, BSP], f32, tag="psel")
                for e in range(E_PAIR):
                    nc.tensor.matmul(psel[:, e, :],
                                     lhsT=selr[:, e * P:(e + 1) * P],
                                     rhs=gT16, start=True, stop=True)
                gTb4 = ypool.tile([P, E_PAIR, BSP], bf16, tag="gTb4")
                nc.vector.tensor_copy(gTb4, psel)

                xcur = xTb
                for l in range(3):
                    par = occ % NBUF
                    # per-expert g-scaled activations
                    xk = xkpool.tile([P, E_PAIR, KSUB, BSP], bf16, tag="xk")
                    for e in range(E_PAIR):
                        nc.vector.tensor_tensor(
                            xk[:, e], xcur,
                            gTb4[:, e:e + 1, :].to_broadcast(
                                (P, KSUB, BSP)),
                            mybir.AluOpType.mult)
                    # matmuls: one psum tile [P, ot, b] (1 bank), 4 groups
                    ps = pspool.tile([P, KSUB, BSP], f32, tag="ps")
                    for ot in range(KSUB):
                        nc.tensor.matmul(ps[:, ot, :], lhsT=bks[l][:, ot, :],
                                         rhs=gT16, start=True, stop=False,
                                         skip_group_check=True)
                        for e in range(E_PAIR):
                            for ks in range(KSUB):
                                nc.tensor.matmul(
                                    ps[:, ot, :],
                                    lhsT=wsl[l][e][:, ks,
                                                   ot * P:(ot + 1) * P],
                                    rhs=xk[:, e, ks, :],
                                    start=False,
                                    stop=(e == E_PAIR - 1 and
                                          ks == KSUB - 1),
                                    skip_group_check=True)
                    # psum -> send buffer (bf16), gated on our sends drained
                    ci = nc.vector.tensor_copy(sendb[:, par], ps)
                    if occ >= NBUF:
                        wj.add(ci, lsem, 16 * (occ - NBUF + 1))
                    # exchange with the SEngine partner (XOR peer 1)
                    rd = [None] * 8
                    rd[1] = (0, 1)
                    nc.gpsimd.remote_dma_broadcast(
                        recvb[:, par], sendb[:, par], rsem[par], lsem,
                        rdests=rd)
                    nc.gpsimd.trigger_dma(1)
                    # reduce: my psum + partner's partial
                    y4 = ypool.tile([P, KSUB, BSP], f32, tag="y4")
                    ri = nc.vector.tensor_tensor(
                        y4, ps, recvb[:, par], mybir.AluOpType.add)
                    wj.add(ri, rsem[par], 2 * (occ // NBUF + 1))
                    occ += 1
                    if l < 2:
                        xcur = _elu_bf16(nc, ypool, y4, [P, KSUB, BSP],
                                         f"ml{l}")
                    else:
                        nc.sync.dma_start(
                            out_d[:], y4.rearrange("p a b -> p (a b)"))
    lower_extended_insts(nc)
    return nc


def prepare_inputs_pair(x, gating_idx, GW1, Gb1, GW2, Gb2, GW3, Gb3,
                        Wk1, bk1, Wk2, bk2, Wk3, bk3):
    import ml_dtypes
    bf = ml_dtypes.bfloat16
    f32 = np.float32
    x = np.asarray(x, f32)
    idx = np.asarray(gating_idx).astype(np.int64)

    Wls = []
    for W in (Wk1, Wk2, Wk3):
        W = np.asarray(W, f32).transpose(0, 2, 1)  # [K, IN, OUT]
        Wp = np.zeros((K, IN_PAD, OUT_PAD), f32)
        Wp[:, :W.shape[1], :W.shape[2]] = W
        # [K, IN, OUT] -> [K, P, KSUB*OUT], partition-major contiguous
        Wls.append(np.ascontiguousarray(
            Wp.reshape(K, KSUB, P, OUT_PAD).swapaxes(1, 2).reshape(
                K, P, KSUB * OUT_PAD)).astype(bf))
    bkls = []
    for bk in (bk1, bk2, bk3):
        bk = np.asarray(bk, f32)
        bkp = np.zeros((K, OUT_PAD), f32)
        bkp[:, :bk.shape[1]] = bk
        bkls.append(bkp.reshape(K, KSUB, P))

    shared = {
        "gw1": np.asarray(GW1, f32).astype(bf),
        "gw2": np.asarray(GW2, f32).astype(bf),
        "gw3": np.asarray(GW3, f32).astype(bf),
        "gb1": np.asarray(Gb1, f32).reshape(GH, 1),
        "gb2": np.asarray(Gb2, f32).reshape(GH, 1),
        "gb3": np.asarray(Gb3, f32).reshape(K, 1),
    }
    xT = np.zeros((IN_PAD, B), f32)
    xT[:IN_DIM] = x.T
    ginT = x[:, idx].T  # [NG, B]

    in_maps = []
    for c in range(N_CORES):
        g, loc = c >> 1, c & 1
        my_experts = range(loc * E_PAIR, (loc + 1) * E_PAIR)
        m = dict(shared)
        xs = xT[:, g * BSP:(g + 1) * BSP]  # [512, 128]
        m["xTb"] = np.ascontiguousarray(
            xs.reshape(KSUB, P, BSP).swapaxes(0, 1)).astype(bf)
        m["ginT"] = np.ascontiguousarray(
            ginT[:, g * BSP:(g + 1) * BSP]).astype(bf)
        selr = np.zeros((K, E_PAIR * P), f32)
        for j, e in enumerate(my_experts):
            selr[e, j * P:(j + 1) * P] = 1.0
        m["selr"] = selr.astype(bf)
        for l in range(3):
            m[f"w{l}"] = np.ascontiguousarray(Wls[l][list(my_experts)])
            bkm = np.zeros_like(bkls[l])
            bkm[list(my_experts)] = bkls[l][list(my_experts)]
            m[f"bk{l}"] = bkm.astype(bf)
        in_maps.append(m)
    return in_maps


def _get_nc(mode):
    repeat = int(os.environ.get("MANN_BENCH_REPEAT", "1"))
    impl = os.environ.get("MANN_IMPL", "repl")
    key = (impl, mode, repeat)
    if key not in _NC_CACHE:
        if impl == "pair":
            _NC_CACHE[key] = _build_pair(repeat)
        else:
            _NC_CACHE[key] = _build(mode, repeat)
    return _NC_CACHE[key]


def _make_emat():
    e = np.zeros((K, K * P), np.float32)
    for j in range(K):
        e[j, j * P:(j + 1) * P] = 1.0
    return e


def prepare_inputs(x, gating_idx, GW1, Gb1, GW2, Gb2, GW3, Gb3,
                   Wk1, bk1, Wk2, bk2, Wk3, bk3, mode):
    if os.environ.get("MANN_IMPL", "repl") == "pair":
        return prepare_inputs_pair(x, gating_idx, GW1, Gb1, GW2, Gb2,
                                   GW3, Gb3, Wk1, bk1, Wk2, bk2, Wk3, bk3)
    wnp = ml_dtypes.bfloat16 if mode == "bf16" else np.float32
    f32 = np.float32
    x = np.asarray(x, f32)
    idx = np.asarray(gating_idx).astype(np.int64)

    xT = np.zeros((IN_PAD, B), f32)
    xT[:IN_DIM] = x.T
    ginT = np.ascontiguousarray(x[:, idx].T)

    w1 = np.zeros((K, IN_PAD, HID), f32)
    w1[:, :IN_DIM] = np.asarray(Wk1, f32).transpose(0, 2, 1)
    w2 = np.ascontiguousarray(np.asarray(Wk2, f32).transpose(0, 2, 1))
    w3 = np.ascontiguousarray(np.asarray(Wk3, f32).transpose(0, 2, 1))

    def _pmajor(w):
        # [K, IN, OUT] -> [K, P, KSUB*OUT]: partition p holds contraction
        # rows {ks*128+p} contiguously (matches the kernel's subtile order).
        kk, ind, outd = w.shape
        return np.ascontiguousarray(
            w.reshape(kk, KSUB, P, outd).swapaxes(1, 2).reshape(
                kk, P, KSUB * outd))

    w1, w2, w3 = _pmajor(w1), _pmajor(w2), _pmajor(w3)

    shared = {
        "w0": w1.astype(wnp), "w1": w2.astype(wnp), "w2": w3.astype(wnp),
        "b0": np.asarray(bk1, f32).astype(wnp),
        "b1": np.asarray(bk2, f32).astype(wnp),
        "b2": np.asarray(bk3, f32).astype(wnp),
        "gw1": np.asarray(GW1, f32), "gw2": np.asarray(GW2, f32),
        "gw3": np.asarray(GW3, f32),
        "gb1": np.asarray(Gb1, f32).reshape(GH, 1),
        "gb2": np.asarray(Gb2, f32).reshape(GH, 1),
        "gb3": np.asarray(Gb3, f32).reshape(K, 1),
        "emat": _make_emat(),
    }
    in_maps = []
    for c in range(N_CORES):
        m = dict(shared)
        m["xT"] = np.ascontiguousarray(xT[:, c * BS:(c + 1) * BS])
        m["ginT"] = np.ascontiguousarray(ginT[:, c * BS:(c + 1) * BS])
        in_maps.append(m)
    return in_maps


def kernel(**inputs):
    global LAST_RESULTS
    mode = MM_MODE
    nc = _get_nc(mode)
    in_maps = prepare_inputs(mode=mode, **inputs)
    trace = os.environ.get("MANN_TRACE", "0") == "1"
    kwargs = {}
    if trace:
        kwargs["trace"] = True
    res = run_bass_kernel_spmd(nc, in_maps, core_ids=list(range(N_CORES)),
                               **kwargs)
    LAST_RESULTS = res
    if os.environ.get("MANN_IMPL", "repl") == "pair":
        blocks = []
        for g in range(N_CORES // 2):
            arr = np.asarray(res.results[2 * g]["out"], np.float32)
            yb = arr.reshape(P, KSUB, BSP).transpose(1, 0, 2).reshape(
                OUT_PAD, BSP)[:OUT_DIM]
            blocks.append(yb.T)
        return np.concatenate(blocks, axis=0)
    out = np.concatenate([r["out"] for r in res.results], axis=0)
    return out.astype(np.float32)



# revision 19
# speedup vs baseline: 1.5731x; 1.5731x over previous
"""MANN (phase-blended mixture-of-experts) forward pass on 8 Trainium2 cores.

Strategy (data-parallel, per sharding hint):
  - Shard batch B=512 across 8 cores (64 samples each); replicate all weights.
  - Host-side prep: transpose expert weights to [K, IN, OUT] (so the device
    streams them in natural layout with the contraction dim on partitions),
    pad layer-1 input dim 480 -> 512, pre-gather the gating columns.
  - Device: activations kept transposed [feat, B].  Key algebraic trick:
        y = sum_k g[:,k] * (x @ Wk[k].T)  ==  sum_k ((g[:,k]*x) @ Wk[k].T)
    so scaling the stationary activations by g[:,k] lets all 8 experts x 4
    K-subtiles accumulate into a single PSUM tile per layer.  The blended
    bias g @ bk is one extra small matmul into the same PSUM group.
  - ELU built from primitives: elu(x) = max(x, exp(min(x,0)) - 1).

Modes (MANN_MM_MODE env var, default bf16):
  bf16  - weights cast to bf16 on host (rel err ~3.5e-3 vs the 2e-2 gate):
          halves DMA traffic vs fp32 and runs matmuls at 1 cycle/row.
  fp32  - exact (rel err ~6e-7) but ~3.5x slower (4 cycles/row + 2x DMA).

Weight slabs are stored host-side partition-major ([K, P, KSUB*OUT], 4KB
contiguous per partition) -- this DMA-descriptor layout change took the
8-core steady-state body from ~106us to ~49us (the [ko p] strided layout
was descriptor-/row-thrash-limited, not HBM-bandwidth-limited).

All 8 cores share one trn2 chip (topology trn2.8x1, LNC=1), so replicated
weights stream 8x through one HBM: ~93MB/body at bf16.  An expert-sharded
variant (MANN_IMPL=pair: 4 experts/core, batch across 4 SEngine pairs,
SBUF->SBUF remote_dma partial exchange) would halve that and was fully
built + validated in the multi-core simulator, but still hits an opaque
device-side failure under the axon PJRT path -- kept env-gated off.
(float32r was investigated and rejected: walrus requires an fp32->fp32r
data conversion that the host API does not expose.)
"""

import json
import os

import numpy as np
import ml_dtypes

import concourse.bass as bass
import concourse.bass2jax as bass2jax
import concourse.mybir as mybir
import concourse.tile as tile
from concourse import bass_utils as _bass_utils
from concourse.bass_utils import run_bass_kernel_spmd
from concourse.masks import make_identity


# ---------------------------------------------------------------------------
# Post-scheduling BIR wait injection.
#
# Cross-core remote_dma semaphore increments are invisible to the Tile
# scheduling simulator (single-core, no-exec), so an in-trace wait_ge on a
# remotely-incremented semaphore deadlocks the scheduler.  Instead the trace
# carries no such waits; we record (instruction name -> waits) in a global
# table keyed by a sentinel semaphore name placed in the module, and inject
# the sync_info entries into the serialized BIR right before compilation.
# Tile's own RAW/WAR edges on the remote-DMA descgen instructions (whose
# outs alias the local recv buffers) keep program order correct; the
# injected waits provide the actual data-arrival gate at runtime.
_WJ_TABLES = {}


class WaitInjector:
    def __init__(self, nc):
        import uuid
        self.key = f"wjtag_{uuid.uuid4().hex[:12]}"
        nc.alloc_semaphore(name=self.key)  # sentinel: marks the module
        self.table = _WJ_TABLES.setdefault(self.key, {})

    def add(self, inst, sem, value):
        self.table.setdefault(inst.ins.name, []).append(
            (sem.name, sem.num, int(value)))


def _inject_waits(data):
    names = {n for ns in data.get("ant_sem_names", {}).values() for n in ns}
    key = next((n for n in names if n.startswith("wjtag_")), None)
    table = _WJ_TABLES.get(key)
    if not table:
        return data
    hit = 0
    for fn in data.get("functions", []):
        for bb in fn.get("blocks", []):
            for inst in bb.get("instructions", []):
                ws = table.get(inst.get("name"))
                if not ws:
                    continue
                hit += 1
                si = inst.get("sync_info")
                if si is None:
                    si = {"on_update": [], "on_wait": []}
                    inst["sync_info"] = si
                for sname, sid, val in ws:
                    si.setdefault("on_wait", []).append({
                        "ant_name": sname, "id": sid,
                        "sync_type": "semaphore",
                        "wait_mode": "sem-ge-imm", "wait_value": val,
                    })
    assert hit == len(table), (
        f"wait injection: only {hit}/{len(table)} tagged instructions "
        f"found in BIR"
    )
    return data


def _legalize_bir(bir_bytes):
    """This container's walrus build rejects instructions carrying more than
    one semaphore wait (setupSyncWait: "Too many sync wait commands" -- hit by
    the Tile kernel-tail Drain).  Equivalent legal form: hoist all but one
    wait onto single-wait NoOps immediately preceding the instruction on the
    same engine (sequencers process waits in program order)."""
    data = _inject_waits(json.loads(bir_bytes))
    n = 0
    for fn in data.get("functions", []):
        for bb in fn.get("blocks", []):
            out = []
            for inst in bb.get("instructions", []):
                si = inst.get("sync_info")
                waits = si.get("on_wait", []) if si else []
                if len(waits) > 1:
                    for w in waits[:-1]:
                        n += 1
                        out.append({
                            "debug": inst.get("debug", 0),
                            "engine": inst["engine"],
                            "ins": [], "outs": [],
                            "name": f"I-mwfix-{n}",
                            "opcode": "NoOp",
                            "sync_info": {"on_update": [], "on_wait": [w]},
                        })
                    si["on_wait"] = [waits[-1]]
                out.append(inst)
            bb["instructions"] = out
    return json.dumps(data).encode()


_orig_compile_bir_kernel = _bass_utils.compile_bir_kernel


def _patched_compile_bir_kernel(bir_json, tmpdir, neff_name="file.neff"):
    return _orig_compile_bir_kernel(_legalize_bir(bir_json), tmpdir,
                                    neff_name=neff_name)


bass2jax.compile_bir_kernel = _patched_compile_bir_kernel
_bass_utils.compile_bir_kernel = _patched_compile_bir_kernel

B, IN_DIM, OUT_DIM, HID, K, GH, NG = 512, 480, 400, 512, 8, 128, 32
N_CORES = 8
BS = B // N_CORES  # 64 samples per core
IN_PAD = 512       # layer-1 contraction dim padded to 4x128
KSUB = 4           # 512 / 128 contraction subtiles (all layers, post-pad)
OUTS = (HID, HID, OUT_DIM)
P = 128

MM_MODE = os.environ.get("MANN_MM_MODE", "bf16")

# Set to the BassKernelResults of the last run (for test harnesses).
LAST_RESULTS = None

_NC_CACHE = {}


def _elu_from(nc, pool, src_ap, out_shape, tag):
    """elu(src) = max(src, min(exp(src), 1) - 1); src may be PSUM or SBUF.
    3 ops, exp directly from src (activations here are small enough that
    exp cannot overflow fp32).  Returns a new SBUF fp32 tile."""
    f32 = mybir.dt.float32
    texp = pool.tile(out_shape, f32, tag=f"{tag}_exp")
    nc.scalar.activation(texp, src_ap, mybir.ActivationFunctionType.Exp)
    nc.vector.tensor_scalar(texp, texp, 1.0, -1.0, mybir.AluOpType.min,
                            mybir.AluOpType.add)
    y = pool.tile(out_shape, f32, tag=f"{tag}_y")
    nc.vector.tensor_tensor(y, src_ap, texp, mybir.AluOpType.max)
    return y


def _build(mode, repeat=1):
    f32 = mybir.dt.float32
    if mode == "bf16":
        wdt = mybir.dt.bfloat16
        mmdt = mybir.dt.bfloat16
    else:
        wdt = f32
        mmdt = f32

    def mm_ap(ap):
        return ap

    nc = bass.Bass()

    xT_d = nc.dram_tensor("xT", [IN_PAD, BS], f32, kind="ExternalInput")
    ginT_d = nc.dram_tensor("ginT", [NG, BS], f32, kind="ExternalInput")
    # Weight slabs stored host-side as [K, P, KSUB*OUT] (partition-major,
    # 4KB bf16 contiguous per partition) so each expert slab is one DMA of
    # 128 large contiguous descriptors instead of 512 strided 1KB ones.
    w_d = [
        nc.dram_tensor(f"w{l}", [K, P, KSUB * OUTS[l]], wdt,
                       kind="ExternalInput")
        for l in range(3)
    ]
    b_d = [
        nc.dram_tensor(f"b{l}", [K, OUTS[l]], wdt, kind="ExternalInput")
        for l in range(3)
    ]
    gw1_d = nc.dram_tensor("gw1", [NG, GH], f32, kind="ExternalInput")
    gw2_d = nc.dram_tensor("gw2", [GH, GH], f32, kind="ExternalInput")
    gw3_d = nc.dram_tensor("gw3", [GH, K], f32, kind="ExternalInput")
    gb1_d = nc.dram_tensor("gb1", [GH, 1], f32, kind="ExternalInput")
    gb2_d = nc.dram_tensor("gb2", [GH, 1], f32, kind="ExternalInput")
    gb3_d = nc.dram_tensor("gb3", [K, 1], f32, kind="ExternalInput")
    # E[j, e*128 + p] = (j == e): replicates g row e across 128 partitions
    # via matmul E_slice.T @ gT.
    emat_d = nc.dram_tensor("emat", [K, K * P], f32, kind="ExternalInput")
    out_d = nc.dram_tensor("out", [BS, OUT_DIM], f32, kind="ExternalOutput")

    w_bufs = int(os.environ.get("MANN_W_BUFS", "24" if mode == "bf16" else "12"))
    with tile.TileContext(nc) as tc:
        with (
            tc.tile_pool(name="consts", bufs=1) as cpool,
            tc.tile_pool(name="w", bufs=w_bufs) as wpool,
            tc.tile_pool(name="stat", bufs=2) as spool,
            tc.tile_pool(name="xt", bufs=2) as xpool,
            tc.tile_pool(name="y", bufs=2) as ypool,
            tc.tile_pool(name="psy", bufs=2, space="PSUM") as pspool,
            tc.tile_pool(name="pstr", bufs=2, space="PSUM") as ptpool,
            tc.tile_pool(name="psg", bufs=1, space="PSUM") as pgpool,
        ):
            pools = (cpool, wpool, spool, xpool, ypool, pspool, ptpool, pgpool)

            # ---- constants ----
            xt0 = cpool.tile([P, KSUB, BS], f32)
            nc.sync.dma_start(xt0, xT_d.rearrange("(ko p) b -> p ko b", p=P))
            gin = cpool.tile([NG, BS], f32)
            nc.sync.dma_start(gin, ginT_d[:])
            gw1 = cpool.tile([NG, GH], f32)
            nc.sync.dma_start(gw1, gw1_d[:])
            gw2 = cpool.tile([GH, GH], f32)
            nc.sync.dma_start(gw2, gw2_d[:])
            gw3 = cpool.tile([GH, K], f32)
            nc.sync.dma_start(gw3, gw3_d[:])
            gb1 = cpool.tile([GH, 1], f32)
            nc.sync.dma_start(gb1, gb1_d[:])
            gb2 = cpool.tile([GH, 1], f32)
            nc.sync.dma_start(gb2, gb2_d[:])
            gb3 = cpool.tile([K, 1], f32)
            nc.sync.dma_start(gb3, gb3_d[:])
            emat = cpool.tile([K, K * P], f32)
            nc.sync.dma_start(emat, emat_d[:])
            bts = []
            for l in range(3):
                bt = cpool.tile([K, OUTS[l]], wdt, tag=f"b{l}")
                nc.sync.dma_start(bt, b_d[l][:])
                bts.append(bt)
            ident = cpool.tile([BS, BS], f32)
            make_identity(nc, ident)
            consts = (xt0, gin, gw1, gw2, gw3, gb1, gb2, gb3, emat, bts, ident)

            if repeat == 0:
                # no-op baseline for dispatch-overhead measurement
                yo = ypool.tile([BS, OUT_DIM], f32, tag="yo")
                nc.vector.memset(yo, 0.0)
                nc.sync.dma_start(out_d[:], yo)
            for _rep in range(repeat):
                _emit_body(nc, mode, mmdt, mm_ap, wdt, pools, w_d, b_d, out_d,
                           consts, accum=(_rep > 0))

    return nc


def _emit_body(nc, mode, mmdt, mm_ap, wdt, pools, w_d, b_d, out_d, consts,
               accum=False):
    f32 = mybir.dt.float32
    cpool, wpool, spool, xpool, ypool, pspool, ptpool, pgpool = pools
    xt0, gin, gw1, gw2, gw3, gb1, gb2, gb3, emat, bts, ident = consts

    # ---- weight slab DMAs, issued first (DMA is the bottleneck) ----
    wsl = []
    for l in range(3):
        row = []
        for e in range(K):
            t = wpool.tile([P, KSUB, OUTS[l]], wdt, tag="w")
            nc.sync.dma_start(
                t[:, :, : OUTS[l]],
                w_d[l][e].rearrange("p (a b) -> p a b", a=KSUB),
            )
            row.append(t)
        wsl.append(row)

    # ---- gating MLP (fp32, exact) ----
    pg1 = pgpool.tile([GH, BS], f32, tag="psg")
    nc.tensor.matmul(pg1, lhsT=gw1, rhs=gin, start=True, stop=True)
    zg1 = ypool.tile([GH, BS], f32, tag="zg1")
    nc.scalar.activation(zg1, pg1, mybir.ActivationFunctionType.Identity,
                         bias=gb1)
    h1 = _elu_from(nc, ypool, zg1, [GH, BS], "g1")

    pg2 = pgpool.tile([GH, BS], f32, tag="psg")
    nc.tensor.matmul(pg2, lhsT=gw2, rhs=h1, start=True, stop=True)
    zg2 = ypool.tile([GH, BS], f32, tag="zg2")
    nc.scalar.activation(zg2, pg2, mybir.ActivationFunctionType.Identity,
                         bias=gb2)
    h2 = _elu_from(nc, ypool, zg2, [GH, BS], "g2")

    pg3 = pgpool.tile([K, BS], f32, tag="psg")
    nc.tensor.matmul(pg3, lhsT=gw3, rhs=h2, start=True, stop=True)
    gT = ypool.tile([K, BS], f32, tag="gT")
    nc.scalar.activation(gT, pg3, mybir.ActivationFunctionType.Identity,
                         bias=gb3)
    if mode == "bf16":
        gT_mm = ypool.tile([K, BS], mmdt, tag="gTmm")
        nc.vector.tensor_copy(gT_mm, gT)
    else:
        gT_mm = gT

    # replicate g across partitions: gTb[p, e, b] = g[b, e]
    pgt = pgpool.tile([P, K, BS], f32, tag="psgtb")
    for e in range(K):
        nc.tensor.matmul(pgt[:, e, :], lhsT=emat[:, e * P:(e + 1) * P],
                         rhs=gT, start=True, stop=True)
    gTb = ypool.tile([P, K, BS], f32, tag="gTb")
    nc.vector.tensor_copy(gTb, pgt)

    # ---- motion layers ----
    # Each layer's output columns are split into two halves so the DVE/ACT
    # post-processing (ELU) and PE transposes of half 0 overlap the PE
    # matmuls of half 1.
    xt = xt0
    sdt = mmdt if mode == "bf16" else f32
    for l in range(3):
        outl = OUTS[l]
        halves = [(0, 256), (256, outl)]

        # per-expert scaled stationaries: one broadcast mult per (expert,
        # k-half) instead of 32 tiny mults
        xk = spool.tile([P, K, KSUB, BS], sdt, tag="xk")
        for e in range(K):
            gslab = gTb[:, e:e + 1, :].to_broadcast((P, 2, BS))
            nc.vector.tensor_tensor(xk[:, e, 0:2, :], xt[:, 0:2, :], gslab,
                                    mybir.AluOpType.mult)
            nc.vector.tensor_tensor(xk[:, e, 2:4, :], xt[:, 2:4, :], gslab,
                                    mybir.AluOpType.mult)

        use_pair = os.environ.get("MANN_PAIR", "1") == "1"
        pss = []
        for h, (lo, hi) in enumerate(halves):
            if use_pair:
                # Two experts run concurrently in disjoint 64-col groups of
                # the PE array (even experts -> psum rows 0:64, odd ->
                # 64:128 via tile_position=(0,64)); summed on DVE after.
                ps_full = pspool.tile([2 * BS, 256], f32, tag=f"psy{h}",
                                      name=f"psy{l}_{h}")
                psA = ps_full[0:BS, : hi - lo]
                psB = ps_full[BS:2 * BS, : hi - lo]
                nc.tensor.matmul(psA, lhsT=mm_ap(gT_mm),
                                 rhs=mm_ap(bts[l][:, lo:hi]),
                                 start=True, stop=False,
                                 skip_group_check=True)
                for e0 in range(0, K, 2):
                    for ks in range(KSUB):
                        last = (e0 == K - 2 and ks == KSUB - 1)
                        nc.tensor.matmul(
                            psA,
                            lhsT=mm_ap(xk[:, e0, ks, :]),
                            rhs=mm_ap(wsl[l][e0][:, ks, lo:hi]),
                            start=False, stop=last,
                            skip_group_check=True,
                        )
                        nc.tensor.matmul(
                            psB,
                            lhsT=mm_ap(xk[:, e0 + 1, ks, :]),
                            rhs=mm_ap(wsl[l][e0 + 1][:, ks, lo:hi]),
                            start=(e0 == 0 and ks == 0), stop=last,
                            tile_position=(0, BS),
                            skip_group_check=True,
                        )
                pss.append((psA, psB))
            else:
                ps_full = pspool.tile([BS, 256], f32, tag=f"psy{h}",
                                      name=f"psy{l}_{h}")
                ps = ps_full[:, : hi - lo]
                nc.tensor.matmul(ps, lhsT=mm_ap(gT_mm),
                                 rhs=mm_ap(bts[l][:, lo:hi]),
                                 start=True, stop=False)
                for e in range(K):
                    for ks in range(KSUB):
                        nc.tensor.matmul(
                            ps,
                            lhsT=mm_ap(xk[:, e, ks, :]),
                            rhs=mm_ap(wsl[l][e][:, ks, lo:hi]),
                            start=False,
                            stop=(e == K - 1 and ks == KSUB - 1),
                        )
                pss.append((ps, None))

        if l < 2:
            ptr = ptpool.tile([P, KSUB, BS], f32, tag="ptr")
            xt_next = xpool.tile([P, KSUB, BS], f32, tag="xtn")
            for h, (lo, hi) in enumerate(halves):
                psA, psB = pss[h]
                if psB is not None:
                    # DVE may read only one PSUM operand per instruction:
                    # copy psB to SBUF first, then add.
                    zb = ypool.tile([BS, hi - lo], f32, tag=f"zb{h}")
                    nc.vector.tensor_copy(zb, psB)
                    z = ypool.tile([BS, hi - lo], f32, tag=f"z{h}")
                    nc.vector.tensor_tensor(z, psA, zb,
                                            mybir.AluOpType.add)
                    src = z
                else:
                    src = psA
                y = _elu_from(nc, ypool, src, [BS, hi - lo], f"ml{h}")
                for c in range(2):
                    nc.tensor.transpose(ptr[:, 2 * h + c, :],
                                        y[:, c * P:(c + 1) * P], ident)
                nc.vector.tensor_copy(xt_next[:, 2 * h:2 * h + 2, :],
                                      ptr[:, 2 * h:2 * h + 2, :])
            xt = xt_next
        else:
            yo = ypool.tile([BS, OUT_DIM], f32, tag="yo")
            for h, (lo, hi) in enumerate(halves):
                psA, psB = pss[h]
                if psB is not None:
                    zb = ypool.tile([BS, hi - lo], f32, tag=f"zb{h}")
                    nc.vector.tensor_copy(zb, psB)
                    nc.vector.tensor_tensor(yo[:, lo:hi], psA, zb,
                                            mybir.AluOpType.add)
                else:
                    nc.vector.tensor_copy(yo[:, lo:hi], psA)
            if accum:
                # benchmark-repeat builds accumulate so no body is dead code
                nc.gpsimd.dma_start(out_d[:], yo,
                                    accum_op=mybir.AluOpType.add)
            else:
                nc.sync.dma_start(out_d[:], yo)


# ---------------------------------------------------------------------------
# Pair-sharded implementation (MANN_IMPL=pair).
#
# 8 cores = 4 SEngine pairs.  Within a pair, the 8 experts are split 4/4;
# the batch is sharded across pairs (128 samples each).  Each core computes
# the partial blend over its 4 experts for the pair's 128 samples, the two
# partials are exchanged SBUF->SBUF over the intra-SEngine link with one
# remote_dma_broadcast per layer, and both cores reduce + elu.  Activations
# stay in [feature, batch] orientation the whole way (weights are the
# stationary operand) so no transposes are needed.  Total HBM weight
# traffic halves vs full replication (each expert is read by 4 cores, not
# 8).  Cross-core waits are injected post-scheduling (see WaitInjector).
E_PAIR = 4          # experts per core
BSP = 128           # samples per pair
OUT_PAD = 512       # L3 out dim padded 400 -> 512


def _elu_bf16(nc, pool, src_ap, shape, tag):
    """elu with bf16 output tile."""
    f32 = mybir.dt.float32
    texp = pool.tile(shape, f32, tag=f"{tag}_exp")
    nc.scalar.activation(texp, src_ap, mybir.ActivationFunctionType.Exp)
    nc.vector.tensor_scalar(texp, texp, 1.0, -1.0, mybir.AluOpType.min,
                            mybir.AluOpType.add)
    y = pool.tile(shape, mybir.dt.bfloat16, tag=f"{tag}_y")
    nc.vector.tensor_tensor(y, src_ap, texp, mybir.AluOpType.max)
    return y


def _build_pair(repeat=1):
    import concourse.tile as tile_mod
    from concourse import library_config
    from concourse.library_overlay import lower_extended_insts

    f32 = mybir.dt.float32
    bf16 = mybir.dt.bfloat16
    nc = bass.Bass()

    xTb_d = nc.dram_tensor("xTb", [P, KSUB, BSP], bf16, kind="ExternalInput")
    gin_d = nc.dram_tensor("ginT", [NG, BSP], bf16, kind="ExternalInput")
    gw1_d = nc.dram_tensor("gw1", [NG, GH], bf16, kind="ExternalInput")
    gw2_d = nc.dram_tensor("gw2", [GH, GH], bf16, kind="ExternalInput")
    gw3_d = nc.dram_tensor("gw3", [GH, K], bf16, kind="ExternalInput")
    gb1_d = nc.dram_tensor("gb1", [GH, 1], f32, kind="ExternalInput")
    gb2_d = nc.dram_tensor("gb2", [GH, 1], f32, kind="ExternalInput")
    gb3_d = nc.dram_tensor("gb3", [K, 1], f32, kind="ExternalInput")
    selr_d = nc.dram_tensor("selr", [K, E_PAIR * P], bf16,
                            kind="ExternalInput")
    w_d = [nc.dram_tensor(f"w{l}", [E_PAIR, P, KSUB * OUT_PAD], bf16,
                          kind="ExternalInput") for l in range(3)]
    bk_d = [nc.dram_tensor(f"bk{l}", [K, KSUB, P], bf16,
                           kind="ExternalInput") for l in range(3)]
    out_d = nc.dram_tensor("out", [P, KSUB * BSP], f32, kind="ExternalOutput")

    wj = WaitInjector(nc)
    NBUF = 3  # exchange-buffer rotation depth
    rsem = [nc.alloc_semaphore(name=f"xsemr{i}") for i in range(NBUF)]
    lsem = nc.alloc_semaphore(name="xseml")

    w_bufs = int(os.environ.get("MANN_PAIR_WBUFS", "24"))
    with tile_mod.TileContext(nc) as tc:
        with (
            tc.tile_pool(name="consts", bufs=1) as cpool,
            tc.tile_pool(name="w", bufs=w_bufs) as wpool,
            tc.tile_pool(name="xk", bufs=2) as xkpool,
            tc.tile_pool(name="y", bufs=3) as ypool,
            tc.tile_pool(name="ps", bufs=2, space="PSUM") as pspool,
            tc.tile_pool(name="psg", bufs=2, space="PSUM") as pgpool,
        ):
            nc.gpsimd.load_library(library_config.proxy)
            xTb = cpool.tile([P, KSUB, BSP], bf16)
            nc.sync.dma_start(xTb, xTb_d[:])
            gin = cpool.tile([NG, BSP], bf16)
            nc.sync.dma_start(gin, gin_d[:])
            gw1 = cpool.tile([NG, GH], bf16)
            nc.sync.dma_start(gw1, gw1_d[:])
            gw2 = cpool.tile([GH, GH], bf16)
            nc.sync.dma_start(gw2, gw2_d[:])
            gw3 = cpool.tile([GH, K], bf16)
            nc.sync.dma_start(gw3, gw3_d[:])
            gb1 = cpool.tile([GH, 1], f32)
            nc.sync.dma_start(gb1, gb1_d[:])
            gb2 = cpool.tile([GH, 1], f32)
            nc.sync.dma_start(gb2, gb2_d[:])
            gb3 = cpool.tile([K, 1], f32)
            nc.sync.dma_start(gb3, gb3_d[:])
            selr = cpool.tile([K, E_PAIR * P], bf16)
            nc.sync.dma_start(selr, selr_d[:])
            bks = []
            for l in range(3):
                bk = cpool.tile([K, KSUB, P], bf16, tag=f"bk{l}")
                nc.sync.dma_start(bk, bk_d[l][:])
                bks.append(bk)
            xdt = f32 if os.environ.get("MANN_PAIR_XF32", "1") == "1" \
                else bf16
            sendb = cpool.tile([P, NBUF, KSUB, BSP], xdt, name="sendb")
            recvb = cpool.tile([P, NBUF, KSUB, BSP], xdt, name="recvb")

            occ = 0
            eocc = 0
            rsem_uses = [0] * NBUF
            for _rep in range(repeat):
                # --- weight DMAs first (the dominant HBM stream) ---
                wsl = []
                for l in range(3):
                    row = []
                    for e in range(E_PAIR):
                        t = wpool.tile([P, KSUB, OUT_PAD], bf16, tag="w")
                        if os.environ.get("MANN_PAIR_DEBUG", "") != "nowdma":
                            nc.sync.dma_start(
                                t, w_d[l][e].rearrange("p (a b) -> p a b",
                                                       a=KSUB))
                        row.append(t)
                    wsl.append(row)

                # --- gating MLP on the pair's 128 samples (bf16) ---
                pg1 = pgpool.tile([GH, BSP], f32, tag="pg")
                nc.tensor.matmul(pg1, lhsT=gw1, rhs=gin, start=True,
                                 stop=True)
                zg1 = ypool.tile([GH, BSP], f32, tag="zg1")
                nc.scalar.activation(zg1, pg1,
                                     mybir.ActivationFunctionType.Identity,
                                     bias=gb1)
                h1 = _elu_bf16(nc, ypool, zg1, [GH, BSP], "g1")
                pg2 = pgpool.tile([GH, BSP], f32, tag="pg")
                nc.tensor.matmul(pg2, lhsT=gw2, rhs=h1, start=True,
                                 stop=True)
                zg2 = ypool.tile([GH, BSP], f32, tag="zg2")
                nc.scalar.activation(zg2, pg2,
                                     mybir.ActivationFunctionType.Identity,
                                     bias=gb2)
                h2 = _elu_bf16(nc, ypool, zg2, [GH, BSP], "g2")
                pg3 = pgpool.tile([K, BSP], f32, tag="pg")
                nc.tensor.matmul(pg3, lhsT=gw3, rhs=h2, start=True,
                                 stop=True)
                gT16 = ypool.tile([K, BSP], bf16, tag="gT16")
                nc.scalar.activation(gT16, pg3,
                                     mybir.ActivationFunctionType.Identity,
                                     bias=gb3)
                # replicate my 4 experts' g rows across all 128 partitions
                psel = pgpool.tile([P, E_PAIR, BSP], f32, tag="psel")
                for e in range(E_PAIR):
                    nc.tensor.matmul(psel[:, e, :],
                                     lhsT=selr[:, e * P:(e + 1) * P],
                                     rhs=gT16, start=True, stop=True)
                gTb4 = ypool.tile([P, E_PAIR, BSP], bf16, tag="gTb4")
                nc.vector.tensor_copy(gTb4, psel)

                xcur = xTb
                for l in range(3):
                    par = occ % NBUF
                    # per-expert g-scaled activations
                    xk = xkpool.tile([P, E_PAIR, KSUB, BSP], bf16, tag="xk")
                    for e in range(E_PAIR):
                        nc.vector.tensor_tensor(
                            xk[:, e], xcur,
                            gTb4[:, e:e + 1, :].to_broadcast(
                                (P, KSUB, BSP)),
                            mybir.AluOpType.mult)
                    # matmuls: one psum tile [P, ot, b] (1 bank), 4 groups
                    ps = pspool.tile([P, KSUB, BSP], f32, tag="ps")
                    for ot in range(KSUB):
                        nc.tensor.matmul(ps[:, ot, :], lhsT=bks[l][:, ot, :],
                                         rhs=gT16, start=True, stop=False,
                                         skip_group_check=True)
                        for e in range(E_PAIR):
                            for ks in range(KSUB):
                                nc.tensor.matmul(
                                    ps[:, ot, :],
                                    lhsT=wsl[l][e][:, ks,
                                                   ot * P:(ot + 1) * P],
                                    rhs=xk[:, e, ks, :],
                                    start=False,
                                    stop=(e == E_PAIR - 1 and
                                          ks == KSUB - 1),
                                    skip_group_check=True)
                    dbg = os.environ.get("MANN_PAIR_DEBUG", "")
                    exch = (dbg != "noexch") and not (
                        dbg in ("exch0", "nowdma") and l > 0)
                    if dbg == "nowait":
                        exch = (l == 0)
                    # psum -> send buffer (bf16), gated on our sends drained
                    ci = nc.vector.tensor_copy(sendb[:, par], ps)
                    if exch and not dbg and occ >= NBUF:
                        wj.add(ci, lsem, 16 * (occ - NBUF + 1))
                    y4 = ypool.tile([P, KSUB, BSP], f32, tag="y4")
                    if exch:
                        # exchange with the SEngine partner (XOR peer 1)
                        rd = [None] * 8
                        rd[1] = (0, 1)
                        nc.gpsimd.remote_dma_broadcast(
                            recvb[:, par].rearrange("p a b -> p (a b)"),
                            sendb[:, par].rearrange("p a b -> p (a b)"),
                            rsem[par], lsem, rdests=rd)
                        nc.gpsimd.trigger_dma(1)
                        eocc += 1
                        rsem_uses[par] += 1
                        # reduce: my psum + partner's partial
                        ri = nc.vector.tensor_tensor(
                            y4, ps, recvb[:, par], mybir.AluOpType.add)
                        if dbg != "nowait":
                            wj.add(ri, rsem[par], 2 * rsem_uses[par])
                    else:
                        # debug: no exchange -- numerically wrong, but
                        # exercises everything else
                        nc.vector.tensor_tensor(
                            y4, ps, sendb[:, par], mybir.AluOpType.add)
                    occ += 1
                    if l < 2:
                        xcur = _elu_bf16(nc, ypool, y4, [P, KSUB, BSP],
                                         f"ml{l}")
                    else:
                        nc.sync.dma_start(
                            out_d[:], y4.rearrange("p a b -> p (a b)"))
    lower_extended_insts(nc)
    return nc


def prepare_inputs_pair(x, gating_idx, GW1, Gb1, GW2, Gb2, GW3, Gb3,
                        Wk1, bk1, Wk2, bk2, Wk3, bk3):
    import ml_dtypes
    bf = ml_dtypes.bfloat16
    f32 = np.float32
    x = np.asarray(x, f32)
    idx = np.asarray(gating_idx).astype(np.int64)

    Wls = []
    for W in (Wk1, Wk2, Wk3):
        W = np.asarray(W, f32).transpose(0, 2, 1)  # [K, IN, OUT]
        Wp = np.zeros((K, IN_PAD, OUT_PAD), f32)
        Wp[:, :W.shape[1], :W.shape[2]] = W
        # [K, IN, OUT] -> [K, P, KSUB*OUT], partition-major contiguous
        Wls.append(np.ascontiguousarray(
            Wp.reshape(K, KSUB, P, OUT_PAD).swapaxes(1, 2).reshape(
                K, P, KSUB * OUT_PAD)).astype(bf))
    bkls = []
    for bk in (bk1, bk2, bk3):
        bk = np.asarray(bk, f32)
        bkp = np.zeros((K, OUT_PAD), f32)
        bkp[:, :bk.shape[1]] = bk
        bkls.append(bkp.reshape(K, KSUB, P))

    shared = {
        "gw1": np.asarray(GW1, f32).astype(bf),
        "gw2": np.asarray(GW2, f32).astype(bf),
        "gw3": np.asarray(GW3, f32).astype(bf),
        "gb1": np.asarray(Gb1, f32).reshape(GH, 1),
        "gb2": np.asarray(Gb2, f32).reshape(GH, 1),
        "gb3": np.asarray(Gb3, f32).reshape(K, 1),
    }
    xT = np.zeros((IN_PAD, B), f32)
    xT[:IN_DIM] = x.T
    ginT = x[:, idx].T  # [NG, B]

    in_maps = []
    for c in range(N_CORES):
        g, loc = c >> 1, c & 1
        my_experts = range(loc * E_PAIR, (loc + 1) * E_PAIR)
        m = dict(shared)
        xs = xT[:, g * BSP:(g + 1) * BSP]  # [512, 128]
        m["xTb"] = np.ascontiguousarray(
            xs.reshape(KSUB, P, BSP).swapaxes(0, 1)).astype(bf)
        m["ginT"] = np.ascontiguousarray(
            ginT[:, g * BSP:(g + 1) * BSP]).astype(bf)
        selr = np.zeros((K, E_PAIR * P), f32)
        for j, e in enumerate(my_experts):
            selr[e, j * P:(j + 1) * P] = 1.0
        m["selr"] = selr.astype(bf)
        for l in range(3):
            m[f"w{l}"] = np.ascontiguousarray(Wls[l][list(my_experts)])
            bkm = np.zeros_like(bkls[l])
            bkm[list(my_experts)] = bkls[l][list(my_experts)]
            m[f"bk{l}"] = bkm.astype(bf)
        in_maps.append(m)
    return in_maps


def _get_nc(mode):
    repeat = int(os.environ.get("MANN_BENCH_REPEAT", "1"))
    impl = os.environ.get("MANN_IMPL", "repl")
    key = (impl, mode, repeat)
    if key not in _NC_CACHE:
        if impl == "pair":
            _NC_CACHE[key] = _build_pair(repeat)
        else:
            _NC_CACHE[key] = _build(mode, repeat)
    return _NC_CACHE[key]


def _make_emat():
    e = np.zeros((K, K * P), np.float32)
    for j in range(K):
        e[j, j * P:(j + 1) * P] = 1.0
    return e


def prepare_inputs(x, gating_idx, GW1, Gb1, GW2, Gb2, GW3, Gb3,
                   Wk1, bk1, Wk2, bk2, Wk3, bk3, mode):
    if os.environ.get("MANN_IMPL", "repl") == "pair":
        return prepare_inputs_pair(x, gating_idx, GW1, Gb1, GW2, Gb2,
                                   GW3, Gb3, Wk1, bk1, Wk2, bk2, Wk3, bk3)
    wnp = ml_dtypes.bfloat16 if mode == "bf16" else np.float32
    f32 = np.float32
    x = np.asarray(x, f32)
    idx = np.asarray(gating_idx).astype(np.int64)

    xT = np.zeros((IN_PAD, B), f32)
    xT[:IN_DIM] = x.T
    ginT = np.ascontiguousarray(x[:, idx].T)

    w1 = np.zeros((K, IN_PAD, HID), f32)
    w1[:, :IN_DIM] = np.asarray(Wk1, f32).transpose(0, 2, 1)
    w2 = np.ascontiguousarray(np.asarray(Wk2, f32).transpose(0, 2, 1))
    w3 = np.ascontiguousarray(np.asarray(Wk3, f32).transpose(0, 2, 1))

    def _pmajor(w):
        # [K, IN, OUT] -> [K, P, KSUB*OUT]: partition p holds contraction
        # rows {ks*128+p} contiguously (matches the kernel's subtile order).
        kk, ind, outd = w.shape
        return np.ascontiguousarray(
            w.reshape(kk, KSUB, P, outd).swapaxes(1, 2).reshape(
                kk, P, KSUB * outd))

    w1, w2, w3 = _pmajor(w1), _pmajor(w2), _pmajor(w3)

    shared = {
        "w0": w1.astype(wnp), "w1": w2.astype(wnp), "w2": w3.astype(wnp),
        "b0": np.asarray(bk1, f32).astype(wnp),
        "b1": np.asarray(bk2, f32).astype(wnp),
        "b2": np.asarray(bk3, f32).astype(wnp),
        "gw1": np.asarray(GW1, f32), "gw2": np.asarray(GW2, f32),
        "gw3": np.asarray(GW3, f32),
        "gb1": np.asarray(Gb1, f32).reshape(GH, 1),
        "gb2": np.asarray(Gb2, f32).reshape(GH, 1),
        "gb3": np.asarray(Gb3, f32).reshape(K, 1),
        "emat": _make_emat(),
    }
    in_maps = []
    for c in range(N_CORES):
        m = dict(shared)
        m["xT"] = np.ascontiguousarray(xT[:, c * BS:(c + 1) * BS])
        m["ginT"] = np.ascontiguousarray(ginT[:, c * BS:(c + 1) * BS])
        in_maps.append(m)
    return in_maps


def kernel(**inputs):
    global LAST_RESULTS
    mode = MM_MODE
    nc = _get_nc(mode)
    in_maps = prepare_inputs(mode=mode, **inputs)
    trace = os.environ.get("MANN_TRACE", "0") == "1"
    kwargs = {}
    if trace:
        kwargs["trace"] = True
    res = run_bass_kernel_spmd(nc, in_maps, core_ids=list(range(N_CORES)),
                               **kwargs)
    LAST_RESULTS = res
    if os.environ.get("MANN_IMPL", "repl") == "pair":
        blocks = []
        for g in range(N_CORES // 2):
            arr = np.asarray(res.results[2 * g]["out"], np.float32)
            yb = arr.reshape(P, KSUB, BSP).transpose(1, 0, 2).reshape(
                OUT_PAD, BSP)[:OUT_DIM]
            blocks.append(yb.T)
        return np.concatenate(blocks, axis=0)
    out = np.concatenate([r["out"] for r in res.results], axis=0)
    return out.astype(np.float32)

